# revision 4
# baseline (speedup 1.0000x reference)
"""GAT message-passing kernel for trn2, 8-core SPMD.

Strategy:
- Nodes are permuted into (core, window) bins so every 128-dst window has a
  near-equal edge count (greedy by in-degree), then sharded across 8 cores.
- Attention-score algebra is folded into the weights: W_src is augmented with
  H extra columns so one matmul yields xs AND s_src; W_dst collapses to
  [D, H] (xd is only ever reduced against att_dst).
- Per layer: each core computes xs/s_src/s_dst for its windows (node phase,
  interleaved into the PREVIOUS layer's LN/residual pipeline), stages a
  [Ndp, 260-fp16-col] table to DRAM in 12-window chunks, and fires a chunk
  AllGather piece as soon as each chunk is staged (collective overlaps the
  edge phase; only the last small piece is exposed).
- Edge phase: edges sorted by dst window, 128-edge tiles. Per tile one
  indirect-DMA gathers the 128 source rows from the gathered table ([128,1]
  offsets; batched offset APs mis-execute on this hardware). A host-built
  0/1 mask matmul segment-reduces weighted messages + softmax denominators
  into PSUM; the transposed mask (packed in the same 256-col strip) expands
  per-dst scores to edges.
- Softmax without max-subtraction (scores are O(1); ratio is identical).
- Graph pooling via per-window matmul against a (1/cnt) selection matrix;
  partials AllReduced at the end; the small MLP runs replicated, with PE
  transposes (no DRAM round-trips) feeding each stage.
"""
import sys
sys.path.insert(0, '/opt/trn_rl_repo')

import numpy as np
import concourse.bass as bass
from concourse import bacc
import concourse.mybir as mybir
import concourse.tile as tile

F32 = mybir.dt.float32
F16 = mybir.dt.float16
I32 = mybir.dt.int32
AF = mybir.ActivationFunctionType
OP = mybir.AluOpType
AX = mybir.AxisListType

H, C = 4, 64
D = H * C          # 256
LAYERS = 3
NCORES = 8
LEAKY = 0.2
TBL = D + 4        # 260 fp16 cols: xs(256) + s_src(4)
TBLW = TBL // 2    # 130 fp32 words


# ---------------------------------------------------------------- host side

def _balance_perm(col, N, n_cores, Nd, W):
    """Permute node ids so each (core, window) bin has ~equal in-edge count.

    Returns perm with perm[old_id] = new_id; new ids keep the contiguous
    core/window/slot encoding used by the rest of build_meta.
    """
    import heapq
    deg = np.bincount(col, minlength=N)
    tail = Nd - (W - 1) * 128                    # slots in the last window
    caps, bases = [], []
    for c in range(n_cores):
        for w in range(W):
            caps.append(128 if w < W - 1 else tail)
            bases.append(c * Nd + w * 128)
    nbins = len(caps)
    fill = [0] * nbins
    load = [0] * nbins
    heap = [(0, b) for b in range(nbins)]
    heapq.heapify(heap)
    perm = np.empty(N, dtype=np.int64)
    order = np.argsort(-deg, kind='stable')
    for v in order:
        spill = []
        while True:
            e, b = heapq.heappop(heap)
            if fill[b] < caps[b]:
                perm[v] = bases[b] + fill[b]
                fill[b] += 1
                load[b] = e + int(deg[v])
                heapq.heappush(heap, (load[b], b))
                break
            spill.append((e, b))
        for it in spill:
            heapq.heappush(heap, it)
    return perm


def build_meta(x, edge_attr, edge_index, batch, n_cores=NCORES, min_G=1):
    N, FIN = x.shape
    E = edge_index.shape[1]
    G = max(int(batch.max()) + 1 if batch.size else 1, min_G)
    assert N % n_cores == 0
    Nd = N // n_cores
    W = (Nd + 127) // 128          # windows per core
    Ndp = W * 128                  # padded nodes per core

    row = np.asarray(edge_index[0], dtype=np.int64)
    col = np.asarray(edge_index[1], dtype=np.int64)
    ea = np.asarray(edge_attr, dtype=np.float32)
    batch = np.asarray(batch, dtype=np.int64)

    # rebalance nodes across (core, window) bins to equalize edge counts
    perm = _balance_perm(col, N, n_cores, Nd, W)
    inv = np.empty(N, dtype=np.int64)
    inv[perm] = np.arange(N)
    x = np.asarray(x)[inv]
    batch = batch[inv]
    row = perm[row]
    col = perm[col]

    # global padded row id, piece-wise AG layout: pieces at every CH-chunk end
    CHW = 6
    bounds = list(range(0, W, CHW)) + [W]          # window boundaries
    rb = [b * 128 for b in bounds]                  # local row boundaries
    def gpad(r):
        c, loc = np.asarray(r // Nd), np.asarray(r % Nd)
        out = np.zeros_like(np.asarray(r))
        base = 0
        for i in range(len(rb) - 1):
            lo, hi = rb[i], rb[i + 1]
            selp = (loc >= lo) & (loc < hi)
            out = np.where(selp, base + c * (hi - lo) + loc - lo, out)
            base += n_cores * (hi - lo)
        return out

    cores = []
    # per (core, window) edge lists
    for c in range(n_cores):
        sel = (col >= c * Nd) & (col < (c + 1) * Nd)
        r_c = row[sel]
        l_c = col[sel] - c * Nd
        e_c = ea[sel]
        order = np.argsort(l_c, kind='stable')
        r_c, l_c, e_c = r_c[order], l_c[order], e_c[order]
        w_c = l_c // 128
        # start offset of each window's run
        starts = np.searchsorted(w_c, np.arange(W + 1))
        cores.append((r_c, l_c, e_c, starts))

    # common schedule: tiles per window = max over cores
    tiles_w = np.zeros(W, dtype=np.int64)
    for wdx in range(W):
        mx = 1
        for c in range(n_cores):
            starts = cores[c][3]
            ne = starts[wdx + 1] - starts[wdx]
            mx = max(mx, (ne + 127) // 128)
        tiles_w[wdx] = mx
    T = int(tiles_w.sum())
    tile_w0 = np.concatenate([[0], np.cumsum(tiles_w)])  # tile offset per window

    idx_all = np.zeros((n_cores, 128, T), dtype=np.int32)
    masks_all = np.zeros((n_cores, 128, T * 256), dtype=np.float16)
    ea_all = np.zeros((n_cores, 128, T * 2), dtype=np.float16)

    for c in range(n_cores):
        r_c, l_c, e_c, starts = cores[c]
        for wdx in range(W):
            s, e = starts[wdx], starts[wdx + 1]
            ne = e - s
            nt = int(tiles_w[wdx])
            cap = nt * 128
            # padded per-edge arrays for this window
            rr = np.zeros(cap, dtype=np.int64)
            ll = np.full(cap, wdx * 128, dtype=np.int64)   # pad dst -> window base
            eaw = np.zeros((cap, 2), dtype=np.float32)
            valid = np.zeros(cap, dtype=bool)
            rr[:ne] = r_c[s:e]
            ll[:ne] = l_c[s:e]
            eaw[:ne] = e_c[s:e]
            valid[:ne] = True
            slot = ll - wdx * 128
            t0 = tile_w0[wdx]
            for t in range(nt):
                sl = slice(t * 128, (t + 1) * 128)
                idx_all[c, :, t0 + t] = gpad(rr[sl]).astype(np.int32)
                m = np.zeros((128, 128), dtype=np.float16)
                vv = valid[sl]
                m[np.arange(128)[vv], slot[sl][vv]] = np.float16(1.0)
                masks_all[c, :, (t0 + t) * 256:(t0 + t) * 256 + 128] = m
                masks_all[c, :, (t0 + t) * 256 + 128:(t0 + t + 1) * 256] = m.T
                ea_all[c, :, (t0 + t) * 2:(t0 + t + 1) * 2] = eaw[sl].astype(np.float16)

    # pooling selection: [128, W*G] value 1/cnt
    cnt = np.bincount(batch, minlength=G).astype(np.float64)
    cnt = np.maximum(cnt, 1.0)
    pool_all = np.zeros((n_cores, 128, W * G), dtype=np.float32)
    for c in range(n_cores):
        for wdx in range(W):
            base = c * Nd + wdx * 128
            nn = min(128, Nd - wdx * 128)
            if nn <= 0:
                continue
            gs = batch[base:base + nn]
            pool_all[c, np.arange(nn), wdx * G + gs] = (1.0 / cnt[gs])
    pool_all = pool_all.astype(np.float16)

    # x transposed + padded per core: [FIN, Ndp]
    xT = np.zeros((n_cores, FIN, Ndp), dtype=np.float16)
    for c in range(n_cores):
        xT[c, :, :Nd] = np.asarray(x[c * Nd:(c + 1) * Nd], dtype=np.float32).T.astype(np.float16)

    return dict(N=N, FIN=FIN, E=E, G=G, Nd=Nd, Ndp=Ndp, W=W, T=T,
                tiles_w=tiles_w.tolist(), tile_w0=tile_w0.tolist(),
                n_cores=n_cores, idx=idx_all, masks=masks_all,
                ea=ea_all, pool=pool_all, xT=xT)


def _const_flags(params):
    f = {}
    f['b_pre0'] = not np.any(params['b_pre'])
    f['bias_conv0'] = not np.any(params['bias_conv'])
    f['gamma1'] = bool(np.all(params['ln_gamma'] == 1.0))
    f['beta0'] = not np.any(params['ln_beta'])
    pa = params['prelu_a']
    f['prelu_const'] = bool(np.all(pa == pa.flat[0])) and 0.0 <= float(pa.flat[0]) <= 1.0
    f['prelu_val'] = float(pa.flat[0])
    f['b_post1_0'] = not np.any(params['b_post1'])
    f['b_post2_0'] = not np.any(params['b_post2'])
    f['b_risk0'] = not np.any(params['b_risk'])
    return f


# ---------------------------------------------------------------- program

def build_program(meta, flags, dbg=False):
    n_cores = meta['n_cores']
    G, W, T, Ndp, FIN = meta['G'], meta['W'], meta['T'], meta['Ndp'], meta['FIN']
    tiles_w, tile_w0 = meta['tiles_w'], meta['tile_w0']
    Tmax = max(tiles_w)
    KF = FIN // 128   # k-tiles for input features
    assert D % 128 == 0
    KD = D // 128     # 2

    nc = bacc.Bacc('TRN2', target_bir_lowering=False, debug=False,
                   num_devices=n_cores, dynamic_dma_scratch_size=16384)

    # ---- dram inputs
    d_xT = nc.dram_tensor("xT", [FIN, Ndp], F16, kind="ExternalInput")
    d_idx = nc.dram_tensor("idx", [128, T], I32, kind="ExternalInput")
    d_masks = nc.dram_tensor("masks", [128, T * 256], F16, kind="ExternalInput")
    d_ea = nc.dram_tensor("ea", [128, T * 2], F16, kind="ExternalInput")
    d_pool = nc.dram_tensor("pool", [128, W * G], F16, kind="ExternalInput")
    d_wpre = nc.dram_tensor("w_pre", [FIN, D], F16, kind="ExternalInput")
    d_bpre = nc.dram_tensor("b_pre", [1, D], F32, kind="ExternalInput")
    d_wsrc = nc.dram_tensor("w_src", [LAYERS, D, TBL], F16, kind="ExternalInput")
    d_wdst = nc.dram_tensor("w_dst", [LAYERS, D, H], F16, kind="ExternalInput")
    d_wedge = nc.dram_tensor("w_edge", [LAYERS, 2, D], F32, kind="ExternalInput")
    d_aedge = nc.dram_tensor("att_edge", [LAYERS, 1, D], F32, kind="ExternalInput")
    d_bconv = nc.dram_tensor("bias_conv", [LAYERS, 1, D], F32, kind="ExternalInput")
    d_gamma = nc.dram_tensor("ln_gamma", [LAYERS, 1, D], F32, kind="ExternalInput")
    d_beta = nc.dram_tensor("ln_beta", [LAYERS, 1, D], F32, kind="ExternalInput")
    d_prelu = nc.dram_tensor("prelu_a", [LAYERS, 1, D], F32, kind="ExternalInput")
    d_w1 = nc.dram_tensor("w_post1", [D * (LAYERS + 1), D], F16, kind="ExternalInput")
    d_b1 = nc.dram_tensor("b_post1", [1, D], F32, kind="ExternalInput")
    d_w2 = nc.dram_tensor("w_post2", [D, D], F16, kind="ExternalInput")
    d_b2 = nc.dram_tensor("b_post2", [1, D], F32, kind="ExternalInput")
    d_wr = nc.dram_tensor("w_risk", [D, 1], F16, kind="ExternalInput")
    d_br = nc.dram_tensor("b_risk", [1, 1], F32, kind="ExternalInput")
    d_out = nc.dram_tensor("risk", [G, 1], F32, kind="ExternalOutput")
    if dbg:
        d_dbg_h0 = nc.dram_tensor("dbg_h0", [128, D], F32, kind="ExternalOutput")
        d_dbg_tbl = nc.dram_tensor("dbg_tbl", [256, TBLW], F32, kind="ExternalOutput")
        d_dbg_g = nc.dram_tensor("dbg_g", [128, TBLW], F32, kind="ExternalOutput")
        d_dbg_u = nc.dram_tensor("dbg_u", [128, TBL], F32, kind="ExternalOutput")
        d_dbg_t16 = nc.dram_tensor("dbg_t16", [128, D], F32, kind="ExternalOutput")
        d_dbg_pool = nc.dram_tensor("dbg_pool", [32, (LAYERS + 1) * D], F32, kind="ExternalOutput")
        d_dbg_xc = nc.dram_tensor("dbg_xc", [32, (LAYERS + 1) * D], F32, kind="ExternalOutput")
        d_dbg_p1 = nc.dram_tensor("dbg_p1", [32, D], F32, kind="ExternalOutput")
        d_dbg_xct = nc.dram_tensor("dbg_xct", [128, (D * (LAYERS + 1) // 128) * 32], F16, kind="ExternalOutput")
        d_dbg_p2 = nc.dram_tensor("dbg_p2", [32, D], F32, kind="ExternalOutput")

    with tile.TileContext(nc) as tc:
        with (
            tc.tile_pool(name="persist", bufs=1) as pp,
            tc.tile_pool(name="wrep", bufs=2) as wrep,
            tc.tile_pool(name="wts", bufs=2) as wts,
            tc.tile_pool(name="ttile", bufs=3) as ttp,
            tc.tile_pool(name="stage", bufs=2) as stp,
            tc.tile_pool(name="hT", bufs=2) as hTp,
            tc.tile_pool(name="gath", bufs=5) as gp,
            tc.tile_pool(name="maskp", bufs=3) as mp,
            tc.tile_pool(name="mprime", bufs=3) as mpr,
            tc.tile_pool(name="scr", bufs=4) as scr,
            tc.tile_pool(name="scr1", bufs=4) as scr1,
            tc.tile_pool(name="ps_u", bufs=2, space="PSUM") as ps_u,
            tc.tile_pool(name="ps_sb", bufs=1, space="PSUM") as ps_sb,
            tc.tile_pool(name="ps_n", bufs=2, space="PSUM") as ps_n,
            tc.tile_pool(name="ps_pool", bufs=1, space="PSUM") as ps_pool,
            tc.tile_pool(name="ps_t", bufs=1, space="PSUM") as ps_t,
            tc.tile_pool(name="dram", bufs=2, space="DRAM") as dp,
        ):
            # ---------------- persistent tiles
            h_sb = pp.tile([128, W, D], F16, tag="h")
            pass  # t16 allocated after xT16 (shared tag)
            sdst16_sb = pp.tile([128, W * H], F16, tag="sdst16")
            idx_sb = pp.tile([128, T], I32, tag="idx")
            ea_sb = pp.tile([128, T, 2], F16, tag="ea")
            pool_sb = pp.tile([128, W, G], F16, tag="pool")
            pooled_sb = pp.tile([32, LAYERS + 1, D], F32, tag="pooled")
            mstat = pp.tile([128, W], F32, tag="mstat")
            sstat = pp.tile([128, W], F32, tag="sstat")
            rstd_t = pp.tile([128, W], F32, tag="rstd")
            nmrs_t = pp.tile([128, W], F32, tag="nmrs")
            ident = pp.tile([128, 128], F16, tag="ident")
            ones1 = pp.tile([1, 128], F32, tag="ones1")

            from concourse.masks import make_identity
            make_identity(nc, ident[:])
            nc.gpsimd.memset(pooled_sb[:], 0.0)
            nc.gpsimd.memset(ones1[:], 1.0)

            nc.sync.dma_start(out=idx_sb[:], in_=d_idx[:])
            nc.sync.dma_start(out=ea_sb[:], in_=d_ea[:].rearrange("p (t k) -> p t k", k=2))
            nc.sync.dma_start(out=pool_sb[:], in_=d_pool[:].rearrange("p (w g) -> p w g", g=G))

            def bcast_load(pool_, dram_ap, parts, width, dt=F32, tag=None):
                t = pool_.tile([parts, width], dt, tag=tag or "bc")
                nc.sync.dma_start(out=t[:], in_=dram_ap.to_broadcast([parts, width]))
                return t

            # ---------------- hoisted weights: all layers up front
            wpre16 = wts.tile([128, KF, D], F16, tag="wmat")
            nc.scalar.dma_start(out=wpre16[:], in_=d_wpre[:].rearrange("(k p) d -> p k d", p=128))
            bpre_rep = None
            if not flags['b_pre0']:
                bpre_rep = bcast_load(wrep, d_bpre[:], 128, D, tag="bpre")
            ws_all = pp.tile([128, LAYERS, KD, TBL], F16, tag="wsall")
            wd_all = pp.tile([128, LAYERS, KD, H], F16, tag="wdall")
            qrep_all = pp.tile([128, LAYERS, 2, H], F32, tag="qrep")
            for l in range(LAYERS):
                nc.scalar.dma_start(out=ws_all[:, l], in_=d_wsrc[l].rearrange("(k p) d -> p k d", p=128))
                nc.scalar.dma_start(out=wd_all[:, l], in_=d_wdst[l].rearrange("(k p) d -> p k d", p=128))
                aedge_rep = bcast_load(scr1, d_aedge[l], 1, D, tag="aedge")
                for k in range(2):
                    wedge_k = scr1.tile([1, D], F32, tag="wedge")
                    nc.sync.dma_start(out=wedge_k[:], in_=d_wedge[l, k:k + 1, :])
                    nc.vector.tensor_tensor(out=wedge_k[:], in0=wedge_k[:], in1=aedge_rep[:], op=OP.mult)
                    qred_k = scr1.tile([1, H], F32, tag="qred")
                    nc.vector.reduce_sum(out=qred_k[:], in_=wedge_k[:].rearrange("p (h c) -> p h c", h=H), axis=AX.X)
                    qps = ps_t.tile([128, H], F32, tag="tr")
                    nc.tensor.matmul(out=qps[:], lhsT=ones1[:], rhs=qred_k[:], start=True, stop=True)
                    nc.vector.tensor_copy(out=qrep_all[:, l, k, :], in_=qps[:])

            xT16 = pp.tile([128, KF, Ndp], F16, tag="big")
            nc.scalar.dma_start(out=xT16[:], in_=d_xT[:].rearrange("(k p) n -> p k n", p=128))
            t16_sb = pp.tile([128, W, D], F16, tag="big")

            CH = 6
            tbounces = [dp.tile([Ndp, TBLW], F32, tag=f"tb{l}", name=f"tbounce{l}") for l in range(LAYERS)]
            tb_rs = [t[:].rearrange("(w p) c -> p w c", p=128) for t in tbounces]
            tshareds = [nc.dram_tensor(f"tshared_l{l}", [n_cores * Ndp, TBLW], F32,
                                       addr_space="Shared") for l in range(LAYERS)]
            CHW = 6
            wbounds = list(range(0, W, CHW)) + [W]   # chunk window boundaries
            pieces = []                               # (lo_row, hi_row, out_base)
            pb = 0
            for i in range(len(wbounds) - 1):
                lo, hi = wbounds[i] * 128, wbounds[i + 1] * 128
                pieces.append((lo, hi, pb))
                pb += n_cores * (hi - lo)

            def ag_piece(l, i):
                lo, hi, base = pieces[i]
                nc.gpsimd.collective_compute(
                    "AllGather", OP.bypass, replica_groups=[list(range(n_cores))],
                    ins=[tbounces[l][lo:hi, :].opt()],
                    outs=[tshareds[l][base:base + n_cores * (hi - lo), :].opt()])

            def node_sub(l, w, stage, j):
                """Layer-l node work for window w into stage slot j (table row + sdst)."""
                hTw = hTp.tile([128, KD, 128], F16, tag="hT")
                for k in range(KD):
                    tps = ps_t.tile([128, 128], F16, tag="tr")
                    nc.tensor.transpose(out=tps[:], in_=h_sb[:, w, k * 128:(k + 1) * 128], identity=ident[:])
                    nc.scalar.activation(hTw[:, k, :], tps[:], AF.Copy)
                xs_ps = ps_n.tile([128, TBL], F32, tag="node")
                xd_ps = ps_sb.tile([128, H], F32, tag="sdps")
                for k in range(KD):
                    nc.tensor.matmul(out=xs_ps[:], lhsT=hTw[:, k, :], rhs=ws_all[:, l, k, :],
                                     start=(k == 0), stop=(k == KD - 1))
                    nc.tensor.matmul(out=xd_ps[:], lhsT=hTw[:, k, :], rhs=wd_all[:, l, k, :],
                                     start=(k == 0), stop=(k == KD - 1))
                nc.scalar.activation(stage[:, j, :], xs_ps[:], AF.Copy)
                nc.vector.tensor_copy(out=sdst16_sb[:, w * H:(w + 1) * H], in_=xd_ps[:])

            # ---------------- pre phase: h0 = x @ W_pre, x0 pooling, node work l=0
            x0_ps = ps_pool.tile([32, D], F32, tag="gpool")
            stage = None
            for w in range(W):
                if w % CH == 0:
                    stage = stp.tile([128, CH, TBL], F16, tag="st")
                hps = ps_n.tile([128, D], F32, tag="node")
                for k in range(KF):
                    nc.tensor.matmul(out=hps[:], lhsT=xT16[:, k, w * 128:(w + 1) * 128],
                                     rhs=wpre16[:, k, :], start=(k == 0), stop=(k == KF - 1))
                if bpre_rep is not None:
                    nc.vector.tensor_tensor(out=hps[:], in0=hps[:], in1=bpre_rep[:], op=OP.add)
                nc.vector.tensor_copy(out=h_sb[:, w, :], in_=hps[:])
                nc.tensor.matmul(out=x0_ps[:G, :], lhsT=pool_sb[:, w, :], rhs=h_sb[:, w, :],
                                 start=(w == 0), stop=(w == W - 1))
                node_sub(0, w, stage, w % CH)
                if w % CH == CH - 1 or w == W - 1:
                    w0 = (w // CH) * CH
                    nc.sync.dma_start(out=tb_rs[0][:, w0:w + 1, :],
                                      in_=stage[:, :w - w0 + 1, :].bitcast(F32))
                    ag_piece(0, w // CH)
            nc.vector.tensor_copy(out=pooled_sb[:G, 0, :], in_=x0_ps[:G, :])

            # ---------------- layers: edge phase l with node work l+1 interleaved
            for l in range(LAYERS):
                tshared = tshareds[l]
                qrep = qrep_all[:, l]
                lpool_ps = ps_pool.tile([32, D], F32, tag="gpool")
                gamma_rep = beta_rep = prelua_rep = None
                if not flags['gamma1']:
                    gamma_rep = bcast_load(wrep, d_gamma[l], 128, D, tag="gamma")
                if not flags['beta0']:
                    beta_rep = bcast_load(wrep, d_beta[l], 128, D, tag="beta")
                if not flags['prelu_const']:
                    prelua_rep = bcast_load(wrep, d_prelu[l], 128, D, tag="prelua")
                last = (l == LAYERS - 1)

                def post_chunk(w0, w1):
                    sl = slice(w0, w1)
                    stage = None if last else stp.tile([128, CH, TBL], F16, tag="st")
                    mean_t = scr1.tile([128, w1 - w0], F32, tag="mean")
                    nc.scalar.mul(out=mean_t[:], in_=mstat[:, sl], mul=1.0 / D)
                    var_t = scr1.tile([128, w1 - w0], F32, tag="var")
                    nc.vector.tensor_tensor(out=var_t[:], in0=mean_t[:], in1=mean_t[:], op=OP.mult)
                    nc.vector.scalar_tensor_tensor(out=var_t[:], in0=sstat[:, sl], scalar=1.0 / D,
                                                   in1=var_t[:], op0=OP.mult, op1=OP.subtract)
                    nc.vector.tensor_scalar_add(out=var_t[:], in0=var_t[:], scalar1=1e-5)
                    nc.vector.reciprocal(out=var_t[:], in_=var_t[:])
                    nc.scalar.sqrt(out=rstd_t[:, sl], in_=var_t[:])
                    nc.vector.tensor_tensor(out=nmrs_t[:, sl], in0=mean_t[:], in1=rstd_t[:, sl], op=OP.mult)
                    nc.vector.tensor_scalar_mul(out=nmrs_t[:, sl], in0=nmrs_t[:, sl], scalar1=-1.0)
                    for w in range(w0, w1):
                        y = ttp.tile([128, D], F16, tag="y")
                        nc.vector.tensor_scalar(out=y[:], in0=t16_sb[:, w, :],
                                                scalar1=rstd_t[:, w:w + 1],
                                                scalar2=nmrs_t[:, w:w + 1],
                                                op0=OP.mult, op1=OP.add)
                        if gamma_rep is not None:
                            nc.vector.tensor_tensor(out=y[:], in0=y[:], in1=gamma_rep[:], op=OP.mult)
                        if beta_rep is not None:
                            nc.vector.tensor_tensor(out=y[:], in0=y[:], in1=beta_rep[:], op=OP.add)
                        if flags['prelu_const']:
                            nc.vector.scalar_tensor_tensor(out=y[:], in0=y[:], scalar=flags['prelu_val'],
                                                           in1=y[:], op0=OP.mult, op1=OP.max)
                        else:
                            neg = scr.tile([128, D], F32, tag="neg")
                            nc.vector.tensor_scalar_min(out=neg[:], in0=y[:], scalar1=0.0)
                            nc.vector.tensor_scalar_max(out=y[:], in0=y[:], scalar1=0.0)
                            nc.vector.scalar_tensor_tensor(out=neg[:], in0=neg[:], scalar=1.0,
                                                           in1=prelua_rep[:], op0=OP.mult, op1=OP.mult)
                            nc.vector.tensor_tensor(out=y[:], in0=y[:], in1=neg[:], op=OP.add)
                        nc.tensor.matmul(out=lpool_ps[:G, :], lhsT=pool_sb[:, w, :], rhs=y[:],
                                         start=(w == 0), stop=(w == W - 1))
                        if not last:
                            nc.vector.tensor_tensor(out=h_sb[:, w, :], in0=y[:], in1=h_sb[:, w, :], op=OP.add)
                            node_sub(l + 1, w, stage, w - w0)
                    if not last:
                        nc.sync.dma_start(out=tb_rs[l + 1][:, w0:w1, :],
                                          in_=stage[:, :w1 - w0, :].bitcast(F32))

                for w in range(W):
                    nt = tiles_w[w]
                    t0 = tile_w0[w]
                    g = gp.tile([128, Tmax, TBLW], F32, tag="g")
                    for t in range(nt):
                        nc.gpsimd.indirect_dma_start(
                            out=g[:, t, :], out_offset=None,
                            in_=tshared[:],
                            in_offset=bass.IndirectOffsetOnAxis(ap=idx_sb[:, t0 + t:t0 + t + 1], axis=0))
                    g16 = g[:].bitcast(F16)
                    msk = mp.tile([128, Tmax * 256], F16, tag="mask")
                    nc.sync.dma_start(out=msk[:, :nt * 256], in_=d_masks[:, t0 * 256:(t0 + nt) * 256])
                    # s_base = maskT.T @ sdst (per tile) -> [128e, H]
                    sb_ps = ps_sb.tile([128, Tmax * H], F32, tag="sbase")
                    for t in range(nt):
                        nc.tensor.matmul(out=sb_ps[:, t * H:(t + 1) * H],
                                         lhsT=msk[:, t * 256 + 128:(t + 1) * 256],
                                         rhs=sdst16_sb[:, w * H:(w + 1) * H],
                                         start=True, stop=True)
                    # u = s_base + s_src + s_edge (se computed per window)
                    u = scr.tile([128, Tmax, H], F32, tag="u")
                    se_w = scr.tile([128, Tmax, H], F32, tag="sew")
                    nc.vector.tensor_tensor(
                        out=se_w[:, :nt, :], in0=ea_sb[:, t0:t0 + nt, 0:1].to_broadcast([128, nt, H]),
                        in1=qrep[:, 0:1, :].to_broadcast([128, nt, H]), op=OP.mult)
                    nc.vector.tensor_tensor(out=u[:, :nt, :], in0=sb_ps[:].rearrange("p (t h) -> p t h", h=H)[:, :nt, :],
                                            in1=g16[:, :nt, D:TBL], op=OP.add)
                    nc.vector.tensor_tensor(out=u[:, :nt, :], in0=u[:, :nt, :],
                                            in1=se_w[:, :nt, :], op=OP.add)
                    nc.vector.tensor_tensor(
                        out=se_w[:, :nt, :], in0=ea_sb[:, t0:t0 + nt, 1:2].to_broadcast([128, nt, H]),
                        in1=qrep[:, 1:2, :].to_broadcast([128, nt, H]), op=OP.mult)
                    nc.vector.tensor_tensor(out=u[:, :nt, :], in0=u[:, :nt, :],
                                            in1=se_w[:, :nt, :], op=OP.add)
                    # w = exp(leaky_relu(u)) = max(exp(u), exp(0.2u))
                    e1 = scr.tile([128, Tmax, H], F16, tag="e1")
                    e2 = scr.tile([128, Tmax, H], F16, tag="e2")
                    nc.scalar.activation(e1[:, :nt, :], u[:, :nt, :], AF.Exp)
                    nc.scalar.activation(e2[:, :nt, :], u[:, :nt, :], AF.Exp, scale=LEAKY)
                    w16 = scr.tile([128, Tmax, H], F16, tag="w16")
                    nc.vector.tensor_tensor(out=w16[:, :nt, :], in0=e1[:, :nt, :], in1=e2[:, :nt, :], op=OP.max)
                    # M' = [xs * w (per head), w]
                    mpr_t = mpr.tile([128, Tmax, TBL], F16, tag="mp")
                    nc.vector.tensor_tensor(
                        out=mpr_t[:, :nt, 0:D].rearrange("p t (h c) -> p t h c", h=H),
                        in0=g16[:, :nt, 0:D].rearrange("p t (h c) -> p t h c", h=H),
                        in1=w16[:, :nt, :].rearrange("p t (h o) -> p t h o", o=1).to_broadcast([128, nt, H, C]),
                        op=OP.mult)
                    nc.vector.tensor_copy(out=mpr_t[:, :nt, D:TBL], in_=w16[:, :nt, :])
                    # U accumulation
                    u_ps = ps_u.tile([128, TBL], F32, tag="U")
                    for t in range(nt):
                        nc.tensor.matmul(out=u_ps[:], lhsT=msk[:, t * 256:t * 256 + 128],
                                         rhs=mpr_t[:, t, :], start=(t == 0), stop=(t == nt - 1))
                    # z-divide + stats
                    zrec = scr.tile([128, H], F32, tag="zrec")
                    nc.vector.tensor_scalar_add(out=zrec[:], in0=u_ps[:, D:TBL], scalar1=1e-16)
                    nc.vector.reciprocal(out=zrec[:], in_=zrec[:])
                    wt = scr.tile([128, D], F32, tag="wt")
                    nc.vector.tensor_tensor(
                        out=wt[:].rearrange("p (h c) -> p h c", h=H),
                        in0=u_ps[:, 0:D].rearrange("p (h c) -> p h c", h=H),
                        in1=zrec[:].rearrange("p (h o) -> p h o", o=1).to_broadcast([128, H, C]),
                        op=OP.mult)
                    if not flags['bias_conv0']:
                        bconv_rep = bcast_load(wrep, d_bconv[l], 128, D, tag="bconv")
                        nc.vector.tensor_tensor(out=wt[:], in0=wt[:], in1=bconv_rep[:], op=OP.add)
                    # mean/sq-sum stats + fp16 copy
                    nc.scalar.activation(t16_sb[:, w, :], wt[:], AF.Copy,
                                         accum_out=mstat[:, w:w + 1])
                    sqt = scr.tile([128, D], F16, tag="sqt")
                    nc.scalar.activation(sqt[:], t16_sb[:, w, :], AF.Square,
                                         accum_out=sstat[:, w:w + 1])
                    if (w + 1) % CH == 0 or w == W - 1:
                        post_chunk((w // CH) * CH, w + 1)
                        if not last:
                            ag_piece(l + 1, w // CH)
                nc.vector.tensor_copy(out=pooled_sb[:G, l + 1, :], in_=lpool_ps[:G, :])

            # ---------------- AllReduce pooled partials + MLP
            FD = (LAYERS + 1) * D     # 1024 flat features per graph
            pr_in = dp.tile([32, FD], F32, tag="prin")
            pr_red = nc.dram_tensor("pr_red_sh", [32, FD], F32,
                                    addr_space="Shared")
            nc.sync.dma_start(out=pr_in[:], in_=pooled_sb[:].rearrange("p l d -> p (l d)"))
            nc.gpsimd.collective_compute(
                "AllReduce", OP.add, replica_groups=[list(range(n_cores))],
                ins=[pr_in[:].opt()], outs=[pr_red[:].opt()])
            xsum = pp.tile([32, FD], F32, tag="xsum")
            nc.sync.dma_start(out=xsum[:], in_=pr_red[:])
            xc16 = pp.tile([32, FD], F16, tag="xc16")
            nc.scalar.activation(xc16[:], xsum[:], AF.Copy)
            K1 = FD // 128
            xcT = wts.tile([128, K1, 32], F16, tag="xcT")
            for k in range(K1):
                tps = ps_t.tile([128, 32], F16, tag="tr")
                nc.tensor.transpose(out=tps[:, :32], in_=xc16[:32, k * 128:(k + 1) * 128], identity=ident[:32, :32])
                nc.scalar.activation(xcT[:, k, :], tps[:, :32], AF.Copy)

            if dbg:
                nc.sync.dma_start(out=d_dbg_xc[:], in_=xc[:].rearrange("p l d -> p (l d)"))
            w1_sb = wts.tile([128, K1, D], F16, tag="wmlp")
            nc.scalar.dma_start(out=w1_sb[:], in_=d_w1[:].rearrange("(k p) d -> p k d", p=128))
            p1_ps = ps_n.tile([32, D], F32, tag="node")
            for k in range(K1):
                nc.tensor.matmul(out=p1_ps[:G, :], lhsT=xcT[:, k, :G], rhs=w1_sb[:, k, :],
                                 start=(k == 0), stop=(k == K1 - 1))
            p1 = pp.tile([32, D], F16, tag="p1")
            if not flags['b_post1_0']:
                b1_rep = bcast_load(wrep, d_b1[:], 32, D, tag="b1")
                nc.vector.tensor_tensor(out=p1_ps[:G, :], in0=p1_ps[:G, :], in1=b1_rep[:G, :], op=OP.add)
            nc.scalar.activation(p1[:G, :], p1_ps[:G, :], AF.Relu)

            if dbg:
                nc.sync.dma_start(out=d_dbg_p1[:G, :], in_=p1[:G, :])
                nc.sync.dma_start(out=d_dbg_xct[:], in_=xcT[:].rearrange("p k g -> p (k g)"))
            p1T = wts.tile([128, KD, 32], F16, tag="p1T")
            for k in range(KD):
                tps = ps_t.tile([128, 32], F16, tag="tr")
                nc.tensor.transpose(out=tps[:, :G], in_=p1[:G, k * 128:(k + 1) * 128], identity=ident[:G, :G])
                nc.scalar.activation(p1T[:, k, :G], tps[:, :G], AF.Copy)
            w2_sb = wts.tile([128, KD, D], F16, tag="wmlp")
            nc.scalar.dma_start(out=w2_sb[:], in_=d_w2[:].rearrange("(k p) d -> p k d", p=128))
            p2_ps = ps_n.tile([32, D], F32, tag="node")
            for k in range(KD):
                nc.tensor.matmul(out=p2_ps[:G, :], lhsT=p1T[:, k, :G], rhs=w2_sb[:, k, :],
                                 start=(k == 0), stop=(k == KD - 1))
            p2 = pp.tile([32, D], F16, tag="p2")
            if not flags['b_post2_0']:
                b2_rep = bcast_load(wrep, d_b2[:], 32, D, tag="b2")
                nc.vector.tensor_tensor(out=p2_ps[:G, :], in0=p2_ps[:G, :], in1=b2_rep[:G, :], op=OP.add)
            nc.scalar.activation(p2[:G, :], p2_ps[:G, :], AF.Relu)

            if dbg:
                nc.sync.dma_start(out=d_dbg_p2[:], in_=p2[:])
            p2T = wts.tile([128, KD, 32], F16, tag="p2T")
            for k in range(KD):
                tps = ps_t.tile([128, 32], F16, tag="tr")
                nc.tensor.transpose(out=tps[:, :G], in_=p2[:G, k * 128:(k + 1) * 128], identity=ident[:G, :G])
                nc.scalar.activation(p2T[:, k, :G], tps[:, :G], AF.Copy)
            wr_sb = wts.tile([128, KD, 1], F16, tag="wmlp")
            nc.scalar.dma_start(out=wr_sb[:], in_=d_wr[:].rearrange("(k p) d -> p k d", p=128))
            r_ps = ps_n.tile([32, 1], F32, tag="node")
            for k in range(KD):
                nc.tensor.matmul(out=r_ps[:G, :], lhsT=p2T[:, k, :G], rhs=wr_sb[:, k, :],
                                 start=(k == 0), stop=(k == KD - 1))
            risk_sb = pp.tile([32, 1], F32, tag="risk")
            if not flags['b_risk0']:
                br_rep = bcast_load(wrep, d_br[:], 32, 1, tag="br")
                nc.vector.tensor_tensor(out=r_ps[:G, :], in0=r_ps[:G, :], in1=br_rep[:G, :], op=OP.add)
            nc.vector.tensor_copy(out=risk_sb[:G, :], in_=r_ps[:G, :])
            nc.sync.dma_start(out=d_out[:], in_=risk_sb[:G, :])

    nc.finalize()
    return nc


def build_in_maps(meta, params):
    n_cores = meta['n_cores']
    f32 = lambda a: np.ascontiguousarray(np.asarray(a, dtype=np.float32))
    # augmented W_src: extra H columns computing s_src = (xs * att_src).sum per head
    Ws = f32(params['W_src'])          # [L, D, D]
    Wd = f32(params['W_dst'])          # [L, D, D]
    As = f32(params['att_src'])        # [L, H, C]
    Ad = f32(params['att_dst'])        # [L, H, C]
    ws_aug = np.zeros((LAYERS, D, TBL), dtype=np.float32)
    wd_small = np.zeros((LAYERS, D, H), dtype=np.float32)
    for l in range(LAYERS):
        ws_aug[l, :, :D] = Ws[l]
        for h in range(H):
            ws_aug[l, :, D + h] = Ws[l][:, h * C:(h + 1) * C] @ As[l, h]
            wd_small[l, :, h] = Wd[l][:, h * C:(h + 1) * C] @ Ad[l, h]
    shared = {
        "w_pre": f32(params['W_pre']).astype(np.float16),
        "b_pre": f32(params['b_pre']).reshape(1, D),
        "w_src": ws_aug.astype(np.float16),
        "w_dst": wd_small.astype(np.float16),
        "w_edge": f32(params['W_edge']),
        "att_edge": f32(params['att_edge']).reshape(LAYERS, 1, D),
        "bias_conv": f32(params['bias_conv']).reshape(LAYERS, 1, D),
        "ln_gamma": f32(params['ln_gamma']).reshape(LAYERS, 1, D),
        "ln_beta": f32(params['ln_beta']).reshape(LAYERS, 1, D),
        "prelu_a": f32(params['prelu_a']).reshape(LAYERS, 1, D),
        "w_post1": f32(params['W_post1']).astype(np.float16),
        "b_post1": f32(params['b_post1']).reshape(1, D),
        "w_post2": f32(params['W_post2']).astype(np.float16),
        "b_post2": f32(params['b_post2']).reshape(1, D),
        "w_risk": f32(params['W_risk']).astype(np.float16),
        "b_risk": f32(params['b_risk']).reshape(1, 1),
    }
    in_maps = []
    for c in range(n_cores):
        m = dict(shared)
        m["xT"] = meta['xT'][c]
        m["idx"] = meta['idx'][c]
        m["masks"] = meta['masks'][c]
        m["ea"] = meta['ea'][c]
        m["pool"] = meta['pool'][c]
        in_maps.append(m)
    return in_maps


PARAM_KEYS = ['W_pre', 'b_pre', 'W_src', 'W_dst', 'W_edge', 'att_src', 'att_dst',
              'att_edge', 'bias_conv', 'ln_gamma', 'ln_beta', 'prelu_a',
              'W_post1', 'b_post1', 'W_post2', 'b_post2', 'W_risk', 'b_risk']


def prepare(dbg=False, min_G=1, **inputs):
    """Returns (nc, in_maps, G)."""
    params = {k: np.asarray(inputs[k]) for k in PARAM_KEYS}
    meta = build_meta(np.asarray(inputs['x']), np.asarray(inputs['edge_attr']),
                      np.asarray(inputs['edge_index']), np.asarray(inputs['batch']),
                      min_G=min_G)
    flags = _const_flags(params)
    nc = build_program(meta, flags, dbg=dbg)
    in_maps = build_in_maps(meta, params)
    return nc, in_maps, meta['G']


def kernel(**inputs):
    # the reference pools into G=25 graphs regardless of batch contents
    from concourse.bass_utils import run_bass_kernel_spmd
    nc, in_maps, G = prepare(min_G=25, **inputs)
    res = run_bass_kernel_spmd(nc, in_maps, core_ids=list(range(NCORES)))
    return np.asarray(res.results[0]["risk"], dtype=np.float32)


if __name__ == "__main__":
    pass



# revision 5
# speedup vs baseline: 1.4997x; 1.4997x over previous
"""GAT message-passing kernel for trn2, 8-core SPMD.

Strategy:
- Nodes are permuted into (core, window) bins so every 128-dst window has a
  near-equal edge count (greedy by in-degree), then sharded across 8 cores.
- Attention-score algebra is folded into the weights: W_src is augmented with
  H extra columns so one matmul yields xs AND s_src; W_dst collapses to
  [D, H] (xd is only ever reduced against att_dst).
- Per layer: each core computes xs/s_src/s_dst for its windows (node phase,
  interleaved into the PREVIOUS layer's LN/residual pipeline), stages a
  [Ndp, 260-fp16-col] table to DRAM in 12-window chunks, and fires a chunk
  AllGather piece as soon as each chunk is staged (collective overlaps the
  edge phase; only the last small piece is exposed).
- Edge phase: edges sorted by dst window, 128-edge tiles. Per tile one
  indirect-DMA gathers the 128 source rows from the gathered table ([128,1]
  offsets; batched offset APs mis-execute on this hardware). A host-built
  0/1 mask matmul segment-reduces weighted messages + softmax denominators
  into PSUM; the transposed mask (packed in the same 256-col strip) expands
  per-dst scores to edges.
- Softmax without max-subtraction (scores are O(1); ratio is identical).
- Graph pooling via per-window matmul against a (1/cnt) selection matrix;
  partials AllReduced at the end; the small MLP runs replicated, with PE
  transposes (no DRAM round-trips) feeding each stage.
"""
import sys
sys.path.insert(0, '/opt/trn_rl_repo')

import numpy as np
import concourse.bass as bass
from concourse import bacc
import concourse.mybir as mybir
import concourse.tile as tile

F32 = mybir.dt.float32
F16 = mybir.dt.float16
I32 = mybir.dt.int32
AF = mybir.ActivationFunctionType
OP = mybir.AluOpType
AX = mybir.AxisListType

H, C = 4, 64
D = H * C          # 256
LAYERS = 3
NCORES = 8
LEAKY = 0.2
TBL = D + 4        # 260 fp16 cols: xs(256) + s_src(4)
TBLW = TBL // 2    # 130 fp32 words


# ---------------------------------------------------------------- host side

def _balance_perm(col, N, n_cores, Nd, W):
    """Permute node ids so each (core, window) bin has ~equal in-edge count.

    Returns perm with perm[old_id] = new_id; new ids keep the contiguous
    core/window/slot encoding used by the rest of build_meta.
    """
    import heapq
    deg = np.bincount(col, minlength=N)
    tail = Nd - (W - 1) * 128                    # slots in the last window
    caps, bases = [], []
    for c in range(n_cores):
        for w in range(W):
            caps.append(128 if w < W - 1 else tail)
            bases.append(c * Nd + w * 128)
    nbins = len(caps)
    fill = [0] * nbins
    load = [0] * nbins
    heap = [(0, b) for b in range(nbins)]
    heapq.heapify(heap)
    perm = np.empty(N, dtype=np.int64)
    order = np.argsort(-deg, kind='stable')
    for v in order:
        spill = []
        while True:
            e, b = heapq.heappop(heap)
            if fill[b] < caps[b]:
                perm[v] = bases[b] + fill[b]
                fill[b] += 1
                load[b] = e + int(deg[v])
                heapq.heappush(heap, (load[b], b))
                break
            spill.append((e, b))
        for it in spill:
            heapq.heappush(heap, it)
    return perm


def build_meta(x, edge_attr, edge_index, batch, n_cores=NCORES, min_G=1):
    N, FIN = x.shape
    E = edge_index.shape[1]
    G = max(int(batch.max()) + 1 if batch.size else 1, min_G)
    assert N % n_cores == 0
    Nd = N // n_cores
    W = (Nd + 127) // 128          # windows per core
    Ndp = W * 128                  # padded nodes per core

    row = np.asarray(edge_index[0], dtype=np.int64)
    col = np.asarray(edge_index[1], dtype=np.int64)
    ea = np.asarray(edge_attr, dtype=np.float32)
    batch = np.asarray(batch, dtype=np.int64)

    # rebalance nodes across (core, window) bins to equalize edge counts
    perm = _balance_perm(col, N, n_cores, Nd, W)
    inv = np.empty(N, dtype=np.int64)
    inv[perm] = np.arange(N)
    x = np.asarray(x)[inv]
    batch = batch[inv]
    row = perm[row]
    col = perm[col]

    # global padded row id, piece-wise AG layout: pieces at every CH-chunk end
    CHW = 6
    bounds = list(range(0, W, CHW)) + [W]          # window boundaries
    rb = [b * 128 for b in bounds]                  # local row boundaries
    def gpad(r):
        c, loc = np.asarray(r // Nd), np.asarray(r % Nd)
        out = np.zeros_like(np.asarray(r))
        base = 0
        for i in range(len(rb) - 1):
            lo, hi = rb[i], rb[i + 1]
            selp = (loc >= lo) & (loc < hi)
            out = np.where(selp, base + c * (hi - lo) + loc - lo, out)
            base += n_cores * (hi - lo)
        return out

    cores = []
    # per (core, window) edge lists
    for c in range(n_cores):
        sel = (col >= c * Nd) & (col < (c + 1) * Nd)
        r_c = row[sel]
        l_c = col[sel] - c * Nd
        e_c = ea[sel]
        order = np.argsort(l_c, kind='stable')
        r_c, l_c, e_c = r_c[order], l_c[order], e_c[order]
        w_c = l_c // 128
        # start offset of each window's run
        starts = np.searchsorted(w_c, np.arange(W + 1))
        cores.append((r_c, l_c, e_c, starts))

    # common schedule: tiles per window = max over cores
    tiles_w = np.zeros(W, dtype=np.int64)
    for wdx in range(W):
        mx = 1
        for c in range(n_cores):
            starts = cores[c][3]
            ne = starts[wdx + 1] - starts[wdx]
            mx = max(mx, (ne + 127) // 128)
        tiles_w[wdx] = mx
    T = int(tiles_w.sum())
    tile_w0 = np.concatenate([[0], np.cumsum(tiles_w)])  # tile offset per window

    idx_all = np.zeros((n_cores, 128, T), dtype=np.int32)
    masks_all = np.zeros((n_cores, 128, T * 256), dtype=np.float16)
    ea_all = np.zeros((n_cores, 128, T * 2), dtype=np.float16)

    for c in range(n_cores):
        r_c, l_c, e_c, starts = cores[c]
        for wdx in range(W):
            s, e = starts[wdx], starts[wdx + 1]
            ne = e - s
            nt = int(tiles_w[wdx])
            cap = nt * 128
            # padded per-edge arrays for this window
            rr = np.zeros(cap, dtype=np.int64)
            ll = np.full(cap, wdx * 128, dtype=np.int64)   # pad dst -> window base
            eaw = np.zeros((cap, 2), dtype=np.float32)
            valid = np.zeros(cap, dtype=bool)
            rr[:ne] = r_c[s:e]
            ll[:ne] = l_c[s:e]
            eaw[:ne] = e_c[s:e]
            valid[:ne] = True
            slot = ll - wdx * 128
            t0 = tile_w0[wdx]
            for t in range(nt):
                sl = slice(t * 128, (t + 1) * 128)
                idx_all[c, :, t0 + t] = gpad(rr[sl]).astype(np.int32)
                m = np.zeros((128, 128), dtype=np.float16)
                vv = valid[sl]
                m[np.arange(128)[vv], slot[sl][vv]] = np.float16(1.0)
                masks_all[c, :, (t0 + t) * 256:(t0 + t) * 256 + 128] = m
                masks_all[c, :, (t0 + t) * 256 + 128:(t0 + t + 1) * 256] = m.T
                ea_all[c, :, (t0 + t) * 2:(t0 + t + 1) * 2] = eaw[sl].astype(np.float16)

    # pooling selection: [128, W*G] value 1/cnt
    cnt = np.bincount(batch, minlength=G).astype(np.float64)
    cnt = np.maximum(cnt, 1.0)
    pool_all = np.zeros((n_cores, 128, W * G), dtype=np.float32)
    for c in range(n_cores):
        for wdx in range(W):
            base = c * Nd + wdx * 128
            nn = min(128, Nd - wdx * 128)
            if nn <= 0:
                continue
            gs = batch[base:base + nn]
            pool_all[c, np.arange(nn), wdx * G + gs] = (1.0 / cnt[gs])
    pool_all = pool_all.astype(np.float16)

    # x transposed + padded per core: [FIN, Ndp]
    xT = np.zeros((n_cores, FIN, Ndp), dtype=np.float16)
    for c in range(n_cores):
        xT[c, :, :Nd] = np.asarray(x[c * Nd:(c + 1) * Nd], dtype=np.float32).T.astype(np.float16)

    return dict(N=N, FIN=FIN, E=E, G=G, Nd=Nd, Ndp=Ndp, W=W, T=T,
                tiles_w=tiles_w.tolist(), tile_w0=tile_w0.tolist(),
                n_cores=n_cores, idx=idx_all, masks=masks_all,
                ea=ea_all, pool=pool_all, xT=xT)


def _const_flags(params):
    f = {}
    f['b_pre0'] = not np.any(params['b_pre'])
    f['bias_conv0'] = not np.any(params['bias_conv'])
    f['gamma1'] = bool(np.all(params['ln_gamma'] == 1.0))
    f['beta0'] = not np.any(params['ln_beta'])
    pa = params['prelu_a']
    f['prelu_const'] = bool(np.all(pa == pa.flat[0])) and 0.0 <= float(pa.flat[0]) <= 1.0
    f['prelu_val'] = float(pa.flat[0])
    f['b_post1_0'] = not np.any(params['b_post1'])
    f['b_post2_0'] = not np.any(params['b_post2'])
    f['b_risk0'] = not np.any(params['b_risk'])
    return f


# ---------------------------------------------------------------- program

def build_program(meta, flags, dbg=False):
    n_cores = meta['n_cores']
    G, W, T, Ndp, FIN = meta['G'], meta['W'], meta['T'], meta['Ndp'], meta['FIN']
    tiles_w, tile_w0 = meta['tiles_w'], meta['tile_w0']
    Tmax = max(tiles_w)
    KF = FIN // 128   # k-tiles for input features
    assert D % 128 == 0
    KD = D // 128     # 2

    nc = bacc.Bacc('TRN2', target_bir_lowering=False, debug=False,
                   num_devices=n_cores, dynamic_dma_scratch_size=16384)

    # ---- dram inputs
    d_xT = nc.dram_tensor("xT", [FIN, Ndp], F16, kind="ExternalInput")
    d_idx = nc.dram_tensor("idx", [128, T], I32, kind="ExternalInput")
    d_masks = nc.dram_tensor("masks", [128, T * 256], F16, kind="ExternalInput")
    d_ea = nc.dram_tensor("ea", [128, T * 2], F16, kind="ExternalInput")
    d_pool = nc.dram_tensor("pool", [128, W * G], F16, kind="ExternalInput")
    d_wpre = nc.dram_tensor("w_pre", [FIN, D], F16, kind="ExternalInput")
    d_bpre = nc.dram_tensor("b_pre", [1, D], F32, kind="ExternalInput")
    d_wsrc = nc.dram_tensor("w_src", [LAYERS, D, TBL], F16, kind="ExternalInput")
    d_wdst = nc.dram_tensor("w_dst", [LAYERS, D, H], F16, kind="ExternalInput")
    d_wedge = nc.dram_tensor("w_edge", [LAYERS, 2, D], F32, kind="ExternalInput")
    d_aedge = nc.dram_tensor("att_edge", [LAYERS, 1, D], F32, kind="ExternalInput")
    d_bconv = nc.dram_tensor("bias_conv", [LAYERS, 1, D], F32, kind="ExternalInput")
    d_gamma = nc.dram_tensor("ln_gamma", [LAYERS, 1, D], F32, kind="ExternalInput")
    d_beta = nc.dram_tensor("ln_beta", [LAYERS, 1, D], F32, kind="ExternalInput")
    d_prelu = nc.dram_tensor("prelu_a", [LAYERS, 1, D], F32, kind="ExternalInput")
    d_w1 = nc.dram_tensor("w_post1", [D * (LAYERS + 1), D], F16, kind="ExternalInput")
    d_b1 = nc.dram_tensor("b_post1", [1, D], F32, kind="ExternalInput")
    d_w2 = nc.dram_tensor("w_post2", [D, D], F16, kind="ExternalInput")
    d_b2 = nc.dram_tensor("b_post2", [1, D], F32, kind="ExternalInput")
    d_wr = nc.dram_tensor("w_risk", [D, 1], F16, kind="ExternalInput")
    d_br = nc.dram_tensor("b_risk", [1, 1], F32, kind="ExternalInput")
    d_out = nc.dram_tensor("risk", [G, 1], F32, kind="ExternalOutput")
    if dbg:
        d_dbg_h0 = nc.dram_tensor("dbg_h0", [128, D], F32, kind="ExternalOutput")
        d_dbg_tbl = nc.dram_tensor("dbg_tbl", [256, TBLW], F32, kind="ExternalOutput")
        d_dbg_g = nc.dram_tensor("dbg_g", [128, TBLW], F32, kind="ExternalOutput")
        d_dbg_u = nc.dram_tensor("dbg_u", [128, TBL], F32, kind="ExternalOutput")
        d_dbg_t16 = nc.dram_tensor("dbg_t16", [128, D], F32, kind="ExternalOutput")
        d_dbg_pool = nc.dram_tensor("dbg_pool", [32, (LAYERS + 1) * D], F32, kind="ExternalOutput")
        d_dbg_xc = nc.dram_tensor("dbg_xc", [32, (LAYERS + 1) * D], F32, kind="ExternalOutput")
        d_dbg_p1 = nc.dram_tensor("dbg_p1", [32, D], F32, kind="ExternalOutput")
        d_dbg_xct = nc.dram_tensor("dbg_xct", [128, (D * (LAYERS + 1) // 128) * 32], F16, kind="ExternalOutput")
        d_dbg_p2 = nc.dram_tensor("dbg_p2", [32, D], F32, kind="ExternalOutput")

    with tile.TileContext(nc) as tc:
        with (
            tc.tile_pool(name="persist", bufs=1) as pp,
            tc.tile_pool(name="wrep", bufs=2) as wrep,
            tc.tile_pool(name="wts", bufs=2) as wts,
            tc.tile_pool(name="ttile", bufs=3) as ttp,
            tc.tile_pool(name="stage", bufs=2) as stp,
            tc.tile_pool(name="hT", bufs=2) as hTp,
            tc.tile_pool(name="gath", bufs=5) as gp,
            tc.tile_pool(name="maskp", bufs=3) as mp,
            tc.tile_pool(name="mprime", bufs=3) as mpr,
            tc.tile_pool(name="scr", bufs=4) as scr,
            tc.tile_pool(name="scr1", bufs=4) as scr1,
            tc.tile_pool(name="ps_u", bufs=2, space="PSUM") as ps_u,
            tc.tile_pool(name="ps_sb", bufs=1, space="PSUM") as ps_sb,
            tc.tile_pool(name="ps_n", bufs=2, space="PSUM") as ps_n,
            tc.tile_pool(name="ps_pool", bufs=1, space="PSUM") as ps_pool,
            tc.tile_pool(name="ps_t", bufs=1, space="PSUM") as ps_t,
            tc.tile_pool(name="dram", bufs=2, space="DRAM") as dp,
        ):
            # ---------------- persistent tiles
            h_sb = pp.tile([128, W, D], F16, tag="h")
            pass  # t16 allocated after xT16 (shared tag)
            sdst16_sb = pp.tile([128, W * H], F16, tag="sdst16")
            idx_sb = pp.tile([128, T], I32, tag="idx")
            ea_sb = pp.tile([128, T, 2], F16, tag="ea")
            pool_sb = pp.tile([128, W, G], F16, tag="pool")
            pooled_sb = pp.tile([32, LAYERS + 1, D], F32, tag="pooled")
            mstat = pp.tile([128, W], F32, tag="mstat")
            sstat = pp.tile([128, W], F32, tag="sstat")
            rstd_t = pp.tile([128, W], F32, tag="rstd")
            nmrs_t = pp.tile([128, W], F32, tag="nmrs")
            ident = pp.tile([128, 128], F16, tag="ident")
            ones1 = pp.tile([1, 128], F32, tag="ones1")

            from concourse.masks import make_identity
            make_identity(nc, ident[:])
            nc.gpsimd.memset(pooled_sb[:], 0.0)
            nc.gpsimd.memset(ones1[:], 1.0)

            nc.sync.dma_start(out=idx_sb[:], in_=d_idx[:])
            nc.sync.dma_start(out=ea_sb[:], in_=d_ea[:].rearrange("p (t k) -> p t k", k=2))
            nc.sync.dma_start(out=pool_sb[:], in_=d_pool[:].rearrange("p (w g) -> p w g", g=G))

            def bcast_load(pool_, dram_ap, parts, width, dt=F32, tag=None):
                t = pool_.tile([parts, width], dt, tag=tag or "bc")
                nc.sync.dma_start(out=t[:], in_=dram_ap.to_broadcast([parts, width]))
                return t

            # ---------------- hoisted weights: all layers up front
            wpre16 = wts.tile([128, KF, D], F16, tag="wmat")
            nc.scalar.dma_start(out=wpre16[:], in_=d_wpre[:].rearrange("(k p) d -> p k d", p=128))
            bpre_rep = None
            if not flags['b_pre0']:
                bpre_rep = bcast_load(wrep, d_bpre[:], 128, D, tag="bpre")
            ws_all = pp.tile([128, LAYERS, KD, TBL], F16, tag="wsall")
            wd_all = pp.tile([128, LAYERS, KD, H], F16, tag="wdall")
            qrep_all = pp.tile([128, LAYERS, 2, H], F32, tag="qrep")
            for l in range(LAYERS):
                nc.scalar.dma_start(out=ws_all[:, l], in_=d_wsrc[l].rearrange("(k p) d -> p k d", p=128))
                nc.scalar.dma_start(out=wd_all[:, l], in_=d_wdst[l].rearrange("(k p) d -> p k d", p=128))
                aedge_rep = bcast_load(scr1, d_aedge[l], 1, D, tag="aedge")
                for k in range(2):
                    wedge_k = scr1.tile([1, D], F32, tag="wedge")
                    nc.sync.dma_start(out=wedge_k[:], in_=d_wedge[l, k:k + 1, :])
                    nc.vector.tensor_tensor(out=wedge_k[:], in0=wedge_k[:], in1=aedge_rep[:], op=OP.mult)
                    qred_k = scr1.tile([1, H], F32, tag="qred")
                    nc.vector.reduce_sum(out=qred_k[:], in_=wedge_k[:].rearrange("p (h c) -> p h c", h=H), axis=AX.X)
                    qps = ps_t.tile([128, H], F32, tag="tr")
                    nc.tensor.matmul(out=qps[:], lhsT=ones1[:], rhs=qred_k[:], start=True, stop=True)
                    nc.vector.tensor_copy(out=qrep_all[:, l, k, :], in_=qps[:])

            xT16 = pp.tile([128, KF, Ndp], F16, tag="big")
            nc.scalar.dma_start(out=xT16[:], in_=d_xT[:].rearrange("(k p) n -> p k n", p=128))
            t16_sb = pp.tile([128, W, D], F16, tag="big")

            CH = 6
            tbounces = [dp.tile([Ndp, TBLW], F32, tag=f"tb{l}", name=f"tbounce{l}") for l in range(LAYERS)]
            tb_rs = [t[:].rearrange("(w p) c -> p w c", p=128) for t in tbounces]
            tshareds = [nc.dram_tensor(f"tshared_l{l}", [n_cores * Ndp, TBLW], F32,
                                       addr_space="Shared") for l in range(LAYERS)]
            CHW = 6
            wbounds = list(range(0, W, CHW)) + [W]   # chunk window boundaries
            pieces = []                               # (lo_row, hi_row, out_base)
            pb = 0
            for i in range(len(wbounds) - 1):
                lo, hi = wbounds[i] * 128, wbounds[i + 1] * 128
                pieces.append((lo, hi, pb))
                pb += n_cores * (hi - lo)

            def ag_piece(l, i):
                lo, hi, base = pieces[i]
                nc.gpsimd.collective_compute(
                    "AllGather", OP.bypass, replica_groups=[list(range(n_cores))],
                    ins=[tbounces[l][lo:hi, :].opt()],
                    outs=[tshareds[l][base:base + n_cores * (hi - lo), :].opt()])

            def node_sub(l, w, stage, j):
                """Layer-l node work for window w into stage slot j (table row + sdst)."""
                hTw = hTp.tile([128, KD, 128], F16, tag="hT")
                for k in range(KD):
                    tps = ps_t.tile([128, 128], F16, tag="tr")
                    nc.tensor.transpose(out=tps[:], in_=h_sb[:, w, k * 128:(k + 1) * 128], identity=ident[:])
                    nc.scalar.activation(hTw[:, k, :], tps[:], AF.Copy)
                xs_ps = ps_n.tile([128, TBL], F32, tag="node")
                xd_ps = ps_sb.tile([128, H], F32, tag="sdps")
                for k in range(KD):
                    nc.tensor.matmul(out=xs_ps[:], lhsT=hTw[:, k, :], rhs=ws_all[:, l, k, :],
                                     start=(k == 0), stop=(k == KD - 1))
                    nc.tensor.matmul(out=xd_ps[:], lhsT=hTw[:, k, :], rhs=wd_all[:, l, k, :],
                                     start=(k == 0), stop=(k == KD - 1))
                nc.scalar.activation(stage[:, j, :], xs_ps[:], AF.Copy)
                nc.vector.tensor_copy(out=sdst16_sb[:, w * H:(w + 1) * H], in_=xd_ps[:])

            # ---------------- pre phase: h0 = x @ W_pre, x0 pooling, node work l=0
            x0_ps = ps_pool.tile([32, D], F32, tag="gpool")
            stage = None
            for w in range(W):
                if w % CH == 0:
                    stage = stp.tile([128, CH, TBL], F16, tag="st")
                hps = ps_n.tile([128, D], F32, tag="node")
                for k in range(KF):
                    nc.tensor.matmul(out=hps[:], lhsT=xT16[:, k, w * 128:(w + 1) * 128],
                                     rhs=wpre16[:, k, :], start=(k == 0), stop=(k == KF - 1))
                if bpre_rep is not None:
                    nc.vector.tensor_tensor(out=hps[:], in0=hps[:], in1=bpre_rep[:], op=OP.add)
                nc.vector.tensor_copy(out=h_sb[:, w, :], in_=hps[:])
                nc.tensor.matmul(out=x0_ps[:G, :], lhsT=pool_sb[:, w, :], rhs=h_sb[:, w, :],
                                 start=(w == 0), stop=(w == W - 1))
                node_sub(0, w, stage, w % CH)
                if w % CH == CH - 1 or w == W - 1:
                    w0 = (w // CH) * CH
                    nc.sync.dma_start(out=tb_rs[0][:, w0:w + 1, :],
                                      in_=stage[:, :w - w0 + 1, :].bitcast(F32))
                    ag_piece(0, w // CH)
            nc.vector.tensor_copy(out=pooled_sb[:G, 0, :], in_=x0_ps[:G, :])

            # ---------------- layers: edge phase l with node work l+1 interleaved
            for l in range(LAYERS):
                tshared = tshareds[l]
                qrep = qrep_all[:, l]
                lpool_ps = ps_pool.tile([32, D], F32, tag="gpool")
                gamma_rep = beta_rep = prelua_rep = None
                if not flags['gamma1']:
                    gamma_rep = bcast_load(wrep, d_gamma[l], 128, D, tag="gamma")
                if not flags['beta0']:
                    beta_rep = bcast_load(wrep, d_beta[l], 128, D, tag="beta")
                if not flags['prelu_const']:
                    prelua_rep = bcast_load(wrep, d_prelu[l], 128, D, tag="prelua")
                last = (l == LAYERS - 1)

                def post_chunk(w0, w1):
                    sl = slice(w0, w1)
                    stage = None if last else stp.tile([128, CH, TBL], F16, tag="st")
                    mean_t = scr1.tile([128, w1 - w0], F32, tag="mean")
                    nc.scalar.mul(out=mean_t[:], in_=mstat[:, sl], mul=1.0 / D)
                    var_t = scr1.tile([128, w1 - w0], F32, tag="var")
                    nc.vector.tensor_tensor(out=var_t[:], in0=mean_t[:], in1=mean_t[:], op=OP.mult)
                    nc.vector.scalar_tensor_tensor(out=var_t[:], in0=sstat[:, sl], scalar=1.0 / D,
                                                   in1=var_t[:], op0=OP.mult, op1=OP.subtract)
                    nc.vector.tensor_scalar_add(out=var_t[:], in0=var_t[:], scalar1=1e-5)
                    nc.vector.reciprocal(out=var_t[:], in_=var_t[:])
                    nc.scalar.sqrt(out=rstd_t[:, sl], in_=var_t[:])
                    nc.vector.tensor_tensor(out=nmrs_t[:, sl], in0=mean_t[:], in1=rstd_t[:, sl], op=OP.mult)
                    nc.vector.tensor_scalar_mul(out=nmrs_t[:, sl], in0=nmrs_t[:, sl], scalar1=-1.0)
                    for w in range(w0, w1):
                        y = ttp.tile([128, D], F16, tag="y")
                        nc.vector.tensor_scalar(out=y[:], in0=t16_sb[:, w, :],
                                                scalar1=rstd_t[:, w:w + 1],
                                                scalar2=nmrs_t[:, w:w + 1],
                                                op0=OP.mult, op1=OP.add)
                        if gamma_rep is not None:
                            nc.vector.tensor_tensor(out=y[:], in0=y[:], in1=gamma_rep[:], op=OP.mult)
                        if beta_rep is not None:
                            nc.vector.tensor_tensor(out=y[:], in0=y[:], in1=beta_rep[:], op=OP.add)
                        if flags['prelu_const']:
                            nc.vector.scalar_tensor_tensor(out=y[:], in0=y[:], scalar=flags['prelu_val'],
                                                           in1=y[:], op0=OP.mult, op1=OP.max)
                        else:
                            neg = scr.tile([128, D], F32, tag="neg")
                            nc.vector.tensor_scalar_min(out=neg[:], in0=y[:], scalar1=0.0)
                            nc.vector.tensor_scalar_max(out=y[:], in0=y[:], scalar1=0.0)
                            nc.vector.scalar_tensor_tensor(out=neg[:], in0=neg[:], scalar=1.0,
                                                           in1=prelua_rep[:], op0=OP.mult, op1=OP.mult)
                            nc.vector.tensor_tensor(out=y[:], in0=y[:], in1=neg[:], op=OP.add)
                        nc.tensor.matmul(out=lpool_ps[:G, :], lhsT=pool_sb[:, w, :], rhs=y[:],
                                         start=(w == 0), stop=(w == W - 1))
                        if not last:
                            nc.vector.tensor_tensor(out=h_sb[:, w, :], in0=y[:], in1=h_sb[:, w, :], op=OP.add)
                            node_sub(l + 1, w, stage, w - w0)
                    if not last:
                        nc.sync.dma_start(out=tb_rs[l + 1][:, w0:w1, :],
                                          in_=stage[:, :w1 - w0, :].bitcast(F32))

                uniform = all(t == Tmax for t in tiles_w)
                msk2 = None
                for w in range(W):
                    nt = tiles_w[w]
                    t0 = tile_w0[w]
                    g = gp.tile([128, Tmax, TBLW], F32, tag="g")
                    for t in range(nt):
                        nc.gpsimd.indirect_dma_start(
                            out=g[:, t, :], out_offset=None,
                            in_=tshared[:],
                            in_offset=bass.IndirectOffsetOnAxis(ap=idx_sb[:, t0 + t:t0 + t + 1], axis=0))
                    g16 = g[:].bitcast(F16)
                    if uniform:
                        # one mask DMA serves two windows
                        if w % 2 == 0:
                            msk2 = mp.tile([128, 2 * Tmax * 256], F16, tag="mask")
                            hi = min(2 * Tmax, T - t0)
                            nc.sync.dma_start(out=msk2[:, :hi * 256],
                                              in_=d_masks[:, t0 * 256:(t0 + hi) * 256])
                            msk = msk2[:, :Tmax * 256]
                        else:
                            msk = msk2[:, Tmax * 256:2 * Tmax * 256]
                    else:
                        msk = mp.tile([128, Tmax * 256], F16, tag="mask")
                        nc.sync.dma_start(out=msk[:, :nt * 256], in_=d_masks[:, t0 * 256:(t0 + nt) * 256])
                    # s_base = maskT.T @ sdst (per tile) -> [128e, H]
                    sb_ps = ps_sb.tile([128, Tmax * H], F32, tag="sbase")
                    for t in range(nt):
                        nc.tensor.matmul(out=sb_ps[:, t * H:(t + 1) * H],
                                         lhsT=msk[:, t * 256 + 128:(t + 1) * 256],
                                         rhs=sdst16_sb[:, w * H:(w + 1) * H],
                                         start=True, stop=True)
                    # u = s_base + s_src + s_edge (se computed per window)
                    u = scr.tile([128, Tmax, H], F32, tag="u")
                    se_w = scr.tile([128, Tmax, H], F32, tag="sew")
                    nc.vector.tensor_tensor(
                        out=se_w[:, :nt, :], in0=ea_sb[:, t0:t0 + nt, 0:1].to_broadcast([128, nt, H]),
                        in1=qrep[:, 0:1, :].to_broadcast([128, nt, H]), op=OP.mult)
                    nc.vector.tensor_tensor(out=u[:, :nt, :], in0=sb_ps[:].rearrange("p (t h) -> p t h", h=H)[:, :nt, :],
                                            in1=g16[:, :nt, D:TBL], op=OP.add)
                    nc.vector.tensor_tensor(out=u[:, :nt, :], in0=u[:, :nt, :],
                                            in1=se_w[:, :nt, :], op=OP.add)
                    nc.vector.tensor_tensor(
                        out=se_w[:, :nt, :], in0=ea_sb[:, t0:t0 + nt, 1:2].to_broadcast([128, nt, H]),
                        in1=qrep[:, 1:2, :].to_broadcast([128, nt, H]), op=OP.mult)
                    nc.vector.tensor_tensor(out=u[:, :nt, :], in0=u[:, :nt, :],
                                            in1=se_w[:, :nt, :], op=OP.add)
                    # w = exp(leaky_relu(u)) = max(exp(u), exp(0.2u))
                    e1 = scr.tile([128, Tmax, H], F16, tag="e1")
                    e2 = scr.tile([128, Tmax, H], F16, tag="e2")
                    nc.scalar.activation(e1[:, :nt, :], u[:, :nt, :], AF.Exp)
                    nc.scalar.activation(e2[:, :nt, :], u[:, :nt, :], AF.Exp, scale=LEAKY)
                    w16 = scr.tile([128, Tmax, H], F16, tag="w16")
                    nc.vector.tensor_tensor(out=w16[:, :nt, :], in0=e1[:, :nt, :], in1=e2[:, :nt, :], op=OP.max)
                    # M' = [xs * w (per head), w]
                    mpr_t = mpr.tile([128, Tmax, TBL], F16, tag="mp")
                    nc.vector.tensor_tensor(
                        out=mpr_t[:, :nt, 0:D].rearrange("p t (h c) -> p t h c", h=H),
                        in0=g16[:, :nt, 0:D].rearrange("p t (h c) -> p t h c", h=H),
                        in1=w16[:, :nt, :].rearrange("p t (h o) -> p t h o", o=1).to_broadcast([128, nt, H, C]),
                        op=OP.mult)
                    nc.vector.tensor_copy(out=mpr_t[:, :nt, D:TBL], in_=w16[:, :nt, :])
                    # U accumulation
                    u_ps = ps_u.tile([128, TBL], F32, tag="U")
                    for t in range(nt):
                        nc.tensor.matmul(out=u_ps[:], lhsT=msk[:, t * 256:t * 256 + 128],
                                         rhs=mpr_t[:, t, :], start=(t == 0), stop=(t == nt - 1))
                    # z-divide + stats
                    zrec = scr.tile([128, H], F32, tag="zrec")
                    nc.vector.tensor_scalar_add(out=zrec[:], in0=u_ps[:, D:TBL], scalar1=1e-16)
                    nc.vector.reciprocal(out=zrec[:], in_=zrec[:])
                    wt = scr.tile([128, D], F32, tag="wt")
                    nc.vector.tensor_tensor(
                        out=wt[:].rearrange("p (h c) -> p h c", h=H),
                        in0=u_ps[:, 0:D].rearrange("p (h c) -> p h c", h=H),
                        in1=zrec[:].rearrange("p (h o) -> p h o", o=1).to_broadcast([128, H, C]),
                        op=OP.mult)
                    if not flags['bias_conv0']:
                        bconv_rep = bcast_load(wrep, d_bconv[l], 128, D, tag="bconv")
                        nc.vector.tensor_tensor(out=wt[:], in0=wt[:], in1=bconv_rep[:], op=OP.add)
                    # mean/sq-sum stats + fp16 copy
                    nc.scalar.activation(t16_sb[:, w, :], wt[:], AF.Copy,
                                         accum_out=mstat[:, w:w + 1])
                    sqt = scr.tile([128, D], F16, tag="sqt")
                    nc.scalar.activation(sqt[:], t16_sb[:, w, :], AF.Square,
                                         accum_out=sstat[:, w:w + 1])
                    if (w + 1) % CH == 0 or w == W - 1:
                        post_chunk((w // CH) * CH, w + 1)
                        if not last:
                            ag_piece(l + 1, w // CH)
                nc.vector.tensor_copy(out=pooled_sb[:G, l + 1, :], in_=lpool_ps[:G, :])

            # ---------------- AllReduce pooled partials + MLP
            FD = (LAYERS + 1) * D     # 1024 flat features per graph
            pr_in = dp.tile([32, FD], F32, tag="prin")
            pr_red = nc.dram_tensor("pr_red_sh", [32, FD], F32,
                                    addr_space="Shared")
            nc.sync.dma_start(out=pr_in[:], in_=pooled_sb[:].rearrange("p l d -> p (l d)"))
            nc.gpsimd.collective_compute(
                "AllReduce", OP.add, replica_groups=[list(range(n_cores))],
                ins=[pr_in[:].opt()], outs=[pr_red[:].opt()])
            xsum = pp.tile([32, FD], F32, tag="xsum")
            nc.sync.dma_start(out=xsum[:], in_=pr_red[:])
            xc16 = pp.tile([32, FD], F16, tag="xc16")
            nc.scalar.activation(xc16[:], xsum[:], AF.Copy)
            K1 = FD // 128
            xcT = wts.tile([128, K1, 32], F16, tag="xcT")
            for k in range(K1):
                tps = ps_t.tile([128, 32], F16, tag="tr")
                nc.tensor.transpose(out=tps[:, :32], in_=xc16[:32, k * 128:(k + 1) * 128], identity=ident[:32, :32])
                nc.scalar.activation(xcT[:, k, :], tps[:, :32], AF.Copy)

            if dbg:
                nc.sync.dma_start(out=d_dbg_xc[:], in_=xc[:].rearrange("p l d -> p (l d)"))
            w1_sb = wts.tile([128, K1, D], F16, tag="wmlp")
            nc.scalar.dma_start(out=w1_sb[:], in_=d_w1[:].rearrange("(k p) d -> p k d", p=128))
            p1_ps = ps_n.tile([32, D], F32, tag="node")
            for k in range(K1):
                nc.tensor.matmul(out=p1_ps[:G, :], lhsT=xcT[:, k, :G], rhs=w1_sb[:, k, :],
                                 start=(k == 0), stop=(k == K1 - 1))
            p1 = pp.tile([32, D], F16, tag="p1")
            if not flags['b_post1_0']:
                b1_rep = bcast_load(wrep, d_b1[:], 32, D, tag="b1")
                nc.vector.tensor_tensor(out=p1_ps[:G, :], in0=p1_ps[:G, :], in1=b1_rep[:G, :], op=OP.add)
            nc.scalar.activation(p1[:G, :], p1_ps[:G, :], AF.Relu)

            if dbg:
                nc.sync.dma_start(out=d_dbg_p1[:G, :], in_=p1[:G, :])
                nc.sync.dma_start(out=d_dbg_xct[:], in_=xcT[:].rearrange("p k g -> p (k g)"))
            p1T = wts.tile([128, KD, 32], F16, tag="p1T")
            for k in range(KD):
                tps = ps_t.tile([128, 32], F16, tag="tr")
                nc.tensor.transpose(out=tps[:, :G], in_=p1[:G, k * 128:(k + 1) * 128], identity=ident[:G, :G])
                nc.scalar.activation(p1T[:, k, :G], tps[:, :G], AF.Copy)
            w2_sb = wts.tile([128, KD, D], F16, tag="wmlp")
            nc.scalar.dma_start(out=w2_sb[:], in_=d_w2[:].rearrange("(k p) d -> p k d", p=128))
            p2_ps = ps_n.tile([32, D], F32, tag="node")
            for k in range(KD):
                nc.tensor.matmul(out=p2_ps[:G, :], lhsT=p1T[:, k, :G], rhs=w2_sb[:, k, :],
                                 start=(k == 0), stop=(k == KD - 1))
            p2 = pp.tile([32, D], F16, tag="p2")
            if not flags['b_post2_0']:
                b2_rep = bcast_load(wrep, d_b2[:], 32, D, tag="b2")
                nc.vector.tensor_tensor(out=p2_ps[:G, :], in0=p2_ps[:G, :], in1=b2_rep[:G, :], op=OP.add)
            nc.scalar.activation(p2[:G, :], p2_ps[:G, :], AF.Relu)

            if dbg:
                nc.sync.dma_start(out=d_dbg_p2[:], in_=p2[:])
            p2T = wts.tile([128, KD, 32], F16, tag="p2T")
            for k in range(KD):
                tps = ps_t.tile([128, 32], F16, tag="tr")
                nc.tensor.transpose(out=tps[:, :G], in_=p2[:G, k * 128:(k + 1) * 128], identity=ident[:G, :G])
                nc.scalar.activation(p2T[:, k, :G], tps[:, :G], AF.Copy)
            wr_sb = wts.tile([128, KD, 1], F16, tag="wmlp")
            nc.scalar.dma_start(out=wr_sb[:], in_=d_wr[:].rearrange("(k p) d -> p k d", p=128))
            r_ps = ps_n.tile([32, 1], F32, tag="node")
            for k in range(KD):
                nc.tensor.matmul(out=r_ps[:G, :], lhsT=p2T[:, k, :G], rhs=wr_sb[:, k, :],
                                 start=(k == 0), stop=(k == KD - 1))
            risk_sb = pp.tile([32, 1], F32, tag="risk")
            if not flags['b_risk0']:
                br_rep = bcast_load(wrep, d_br[:], 32, 1, tag="br")
                nc.vector.tensor_tensor(out=r_ps[:G, :], in0=r_ps[:G, :], in1=br_rep[:G, :], op=OP.add)
            nc.vector.tensor_copy(out=risk_sb[:G, :], in_=r_ps[:G, :])
            nc.sync.dma_start(out=d_out[:], in_=risk_sb[:G, :])

    nc.finalize()
    return nc


def build_in_maps(meta, params):
    n_cores = meta['n_cores']
    f32 = lambda a: np.ascontiguousarray(np.asarray(a, dtype=np.float32))
    # augmented W_src: extra H columns computing s_src = (xs * att_src).sum per head
    Ws = f32(params['W_src'])          # [L, D, D]
    Wd = f32(params['W_dst'])          # [L, D, D]
    As = f32(params['att_src'])        # [L, H, C]
    Ad = f32(params['att_dst'])        # [L, H, C]
    ws_aug = np.zeros((LAYERS, D, TBL), dtype=np.float32)
    wd_small = np.zeros((LAYERS, D, H), dtype=np.float32)
    for l in range(LAYERS):
        ws_aug[l, :, :D] = Ws[l]
        for h in range(H):
            ws_aug[l, :, D + h] = Ws[l][:, h * C:(h + 1) * C] @ As[l, h]
            wd_small[l, :, h] = Wd[l][:, h * C:(h + 1) * C] @ Ad[l, h]
    shared = {
        "w_pre": f32(params['W_pre']).astype(np.float16),
        "b_pre": f32(params['b_pre']).reshape(1, D),
        "w_src": ws_aug.astype(np.float16),
        "w_dst": wd_small.astype(np.float16),
        "w_edge": f32(params['W_edge']),
        "att_edge": f32(params['att_edge']).reshape(LAYERS, 1, D),
        "bias_conv": f32(params['bias_conv']).reshape(LAYERS, 1, D),
        "ln_gamma": f32(params['ln_gamma']).reshape(LAYERS, 1, D),
        "ln_beta": f32(params['ln_beta']).reshape(LAYERS, 1, D),
        "prelu_a": f32(params['prelu_a']).reshape(LAYERS, 1, D),
        "w_post1": f32(params['W_post1']).astype(np.float16),
        "b_post1": f32(params['b_post1']).reshape(1, D),
        "w_post2": f32(params['W_post2']).astype(np.float16),
        "b_post2": f32(params['b_post2']).reshape(1, D),
        "w_risk": f32(params['W_risk']).astype(np.float16),
        "b_risk": f32(params['b_risk']).reshape(1, 1),
    }
    in_maps = []
    for c in range(n_cores):
        m = dict(shared)
        m["xT"] = meta['xT'][c]
        m["idx"] = meta['idx'][c]
        m["masks"] = meta['masks'][c]
        m["ea"] = meta['ea'][c]
        m["pool"] = meta['pool'][c]
        in_maps.append(m)
    return in_maps


PARAM_KEYS = ['W_pre', 'b_pre', 'W_src', 'W_dst', 'W_edge', 'att_src', 'att_dst',
              'att_edge', 'bias_conv', 'ln_gamma', 'ln_beta', 'prelu_a',
              'W_post1', 'b_post1', 'W_post2', 'b_post2', 'W_risk', 'b_risk']


def prepare(dbg=False, min_G=1, **inputs):
    """Returns (nc, in_maps, G)."""
    params = {k: np.asarray(inputs[k]) for k in PARAM_KEYS}
    meta = build_meta(np.asarray(inputs['x']), np.asarray(inputs['edge_attr']),
                      np.asarray(inputs['edge_index']), np.asarray(inputs['batch']),
                      min_G=min_G)
    flags = _const_flags(params)
    nc = build_program(meta, flags, dbg=dbg)
    in_maps = build_in_maps(meta, params)
    return nc, in_maps, meta['G']


def kernel(**inputs):
    # the reference pools into G=25 graphs regardless of batch contents
    from concourse.bass_utils import run_bass_kernel_spmd
    nc, in_maps, G = prepare(min_G=25, **inputs)
    res = run_bass_kernel_spmd(nc, in_maps, core_ids=list(range(NCORES)))
    return np.asarray(res.results[0]["risk"], dtype=np.float32)


if __name__ == "__main__":
    pass

